# revision 21
# baseline (speedup 1.0000x reference)
"""Trainium2 Bass kernel for nn_DendriteLayer (topk_masking).

Computation (see reference):
    h  = x @ w_in.T + b_in                    # [B, N_DEND]
    h3 = h.reshape(B, OUT_DIM, DPN)
    out[b,u] = h3[b,u,argmax_d h3[b,u,:]] * w_out[u, argmax_d] + b_out[u]

Sharding: OUT_DIM (and its DPN dendrite groups) split across 8 cores;
x replicated; no cross-core communication. Each core computes a
[B, OUT_DIM/8] slice of the output.

Matmul precision strategy: the argmax makes this kernel require
~fp32-grade h (top-2 dendrite gaps go down to ~1e-6; a flipped argmax
selects a different w_out and produces an O(scale) output error). The
PE's native fp32 mode costs 4 cycles/row and fp32r's reduced precision
(~1e-4) flips hundreds of argmaxes. Instead we split both operands
hi/lo — x = xh(fp16) + xl(bf16 residual), w = wh(fp16) + wl(bf16
residual) — and compute h = xh@wh + xh@wl + xl@wh in three 1-cycle/row
passes accumulated in the same PSUM bank (xl@wl ~ 2^-22 is dropped).
Representation error lands at ~1e-6 on h, which preserves every argmax
on this data. bf16 (not fp16) for the residuals avoids fp16 subnormal
flush for tiny values.

Device layout: batch on partitions, dendrites on the free dim, so the
per-unit max over DPN=16 consecutive dendrites is a free-dim segmented
reduce on the vector engine. w splits are pre-transposed on host to
[IN_DIM, N_DEND] so the contraction dim lands on partitions with
contiguous DMA rows.
"""

import numpy as np

import concourse.bass as bass
import concourse.mybir as mybir
from concourse import tile
from concourse.bass_utils import run_bass_kernel_spmd
from concourse.vector_clock import ScopedClock
from contextlib import ExitStack

# Problem shapes (hardcoded per contract).
B = 256          # batch
K = 1024         # in_dim
OUT_DIM = 2048
DPN = 16
N_CORES = 8
D_SH = (OUT_DIM // N_CORES) * DPN   # 4096 dendrites per core
U_SH = OUT_DIM // N_CORES           # 256 units per core
KT = K // 128                       # 8 k-tiles
DC_W = 512                          # dendrite chunk width (PSUM bank)
DC = D_SH // DC_W                   # 8 chunks
UC = DC_W // DPN                    # 32 units per chunk
NB = B // 128                       # 2 batch tiles
DT = mybir.dt.float32
F16 = mybir.dt.float16
BF16 = mybir.dt.bfloat16
AX = mybir.AxisListType.X


def _patch_tile_tail_drain():
    """Workaround: this container's walrus build rejects >2 semaphore
    waits on one InstDrain ("Too many sync wait commands"). Move the
    TileContext tail-drain waits onto individual SP NOPs (one wait
    each); SP program order keeps the drain equivalent."""
    if getattr(tile.TileContext, "_ant_drain_patched", False):
        return

    def _patched(self, tick_clock, wait_clock):
        nc = self.nc
        probe = nc.sync.nop()
        wait_clock.add_sem_waits(
            probe.ins, ScopedClock({None: tick_clock.global_clock})
        )
        si = probe.ins.sync_info
        waits = list(si.on_wait) if si and si.on_wait else []
        if len(waits) > 1:
            si.on_wait.clear()
            si.on_wait.append(waits[0])
            for w in waits[1:]:
                extra = nc.sync.nop()
                esi = extra.ins.sync_info
                if esi is None:
                    extra.ins.sync_info = mybir.SyncInfo(
                        on_wait=[w], on_update=[]
                    )
                else:
                    esi.on_wait.append(w)
        nc.sync.drain()
        nc.all_engine_barrier()
        assert self.sems is not None
        popped = nc._tile_sem_poison_stack.pop()
        assert popped is self._sem_poison
        nc.clear_and_free_semaphores(list(self.sems.allocated().values()))
        nc.all_engine_barrier()

    tile.TileContext._drain_and_barrier = _patched
    tile.TileContext._ant_drain_patched = True


def _split_excess_waits(nc, limit=1):
    """This container's walrus build rejects instructions carrying more
    than a couple of semaphore waits ("Too many sync wait commands";
    the limit varies per opcode — Matmult fails at 2). Move excess
    waits onto same-engine NoOps inserted immediately before the
    instruction; per-engine program order keeps semantics identical."""
    uid = 0
    for f in nc.m.functions:
        for blk in f.blocks:
            insts = blk.instructions
            out = []
            for inst in insts:
                si = inst.sync_info
                if si is not None and si.on_wait and len(si.on_wait) > limit:
                    waits = list(si.on_wait)
                    excess, keep = waits[:-limit], waits[-limit:]
                    for i in range(0, len(excess), limit):
                        nop = mybir.InstNoOp(
                            name=f"WSPLIT-{uid}", ins=[], outs=[]
                        )
                        uid += 1
                        nop.engine = inst.engine
                        nop.sync_info = mybir.SyncInfo(
                            on_wait=excess[i : i + limit], on_update=[]
                        )
                        out.append(nop)
                    si.on_wait.clear()
                    si.on_wait.extend(keep)
                out.append(inst)
            insts[:] = out


def build_nc(split_waits=True):
    _patch_tile_tail_drain()
    nc = bass.Bass()
    xhT = nc.declare_dram_parameter("xhT", [K, B], F16, isOutput=False)
    xlT = nc.declare_dram_parameter("xlT", [K, B], BF16, isOutput=False)
    whT = nc.declare_dram_parameter("whT", [K, D_SH], F16, isOutput=False)
    wlT = nc.declare_dram_parameter("wlT", [K, D_SH], BF16, isOutput=False)
    bin_ = nc.declare_dram_parameter("bin", [1, D_SH], DT, isOutput=False)
    wout = nc.declare_dram_parameter("wout", [1, D_SH], DT, isOutput=False)
    bout = nc.declare_dram_parameter("bout", [1, U_SH], DT, isOutput=False)
    out = nc.declare_dram_parameter("out", [B, U_SH], DT, isOutput=True)

    with tile.TileContext(nc) as tc, ExitStack() as ctx:
        const = ctx.enter_context(tc.tile_pool(name="const", bufs=1))
        whpool = ctx.enter_context(tc.tile_pool(name="whpool", bufs=3))
        wlpool = ctx.enter_context(tc.tile_pool(name="wlpool", bufs=3))
        hpool = ctx.enter_context(tc.tile_pool(name="hpool", bufs=6))
        epool = ctx.enter_context(tc.tile_pool(name="epool", bufs=6))
        opool = ctx.enter_context(tc.tile_pool(name="opool", bufs=4))
        pspool = ctx.enter_context(
            tc.tile_pool(name="pspool", bufs=7, space="PSUM")
        )
        pswarm = ctx.enter_context(
            tc.tile_pool(name="pswarm", bufs=1, space="PSUM")
        )

        # ---- prologue: k-granular interleave of x splits and chunk-0 w
        # slices across both HWDGE rings, ordered so each k-group's
        # operands (xh, wh | wl | xl) land just before the PE needs
        # them. sync carries wh + xl, scalar carries xh + wl ----
        xhv = xhT.rearrange("(t p) b -> p t b", p=128)
        xlv = xlT.rearrange("(t p) b -> p t b", p=128)
        xh_sb = const.tile([128, KT, B], F16)
        xl_sb = const.tile([128, KT, B], BF16)

        w_tiles = {}

        def load_w(dc, split=1, x_interleave=False):
            dsl = slice(dc * DC_W, (dc + 1) * DC_W)
            whv = whT[:, dsl].rearrange("(t p) d -> p t d", p=128)
            wlv = wlT[:, dsl].rearrange("(t p) d -> p t d", p=128)
            wh_sb = whpool.tile([128, KT, DC_W], F16, name="wh_sb")
            wl_sb = wlpool.tile([128, KT, DC_W], BF16, name="wl_sb")
            step = KT // split
            for i in range(split):
                s = slice(i * step, (i + 1) * step)
                if x_interleave:
                    nc.scalar.dma_start(xh_sb[:, s, :], xhv[:, s, :])
                nc.sync.dma_start(wh_sb[:, s, :], whv[:, s, :])
                nc.scalar.dma_start(wl_sb[:, s, :], wlv[:, s, :])
                if x_interleave:
                    nc.sync.dma_start(xl_sb[:, s, :], xlv[:, s, :])
            w_tiles[dc] = (wh_sb, wl_sb)

        load_w(0, split=8, x_interleave=True)
        load_w(1, split=2)

        # ---- broadcast constants (split across both rings, queued
        # behind the first w chunks so they don't stall the PE) ----
        bin_bc = const.tile([128, D_SH], DT)
        nc.sync.dma_start(bin_bc[:], bin_[0:1, :].broadcast_to([128, D_SH]))

        wout_bc = const.tile([128, D_SH], DT)
        nc.scalar.dma_start(
            wout_bc[:], wout[0:1, :].broadcast_to([128, D_SH])
        )

        bout_bc = const.tile([128, U_SH], DT)
        nc.scalar.dma_start(
            bout_bc[:], bout[0:1, :].broadcast_to([128, U_SH])
        )

        m_t = [const.tile([128, U_SH], DT, name=f"m{b}") for b in range(NB)]

        # ---- PE warmup: tiny matmuls on a zeroed scratch tile keep the
        # PE continuously busy from t~0 so its pstate ramp (full speed
        # after 3us of busy) completes during the DMA prologue instead
        # of during real work. Results land in the first PSUM buffer and
        # are overwritten by the first real matmul (start=True). ----
        warm = const.tile([128, 16], F16, name="warm")
        nc.vector.memset(warm[:], 0.0)
        ps_warm = pswarm.tile([128, 16], DT, name="ps_warm")
        for _ in range(72):
            nc.tensor.matmul(
                ps_warm[0:16, :], warm[:, 0:16], warm[:],
                start=True, stop=True,
            )

        # ---- main stream: 3-pass split matmul + chunked epilogue.
        # The final 512-chunk is processed as 256+128+128-wide tiles so
        # the tail (the last tile's serial DVE chain) is shorter. ----
        plan = [(dc, dc * DC_W, DC_W) for dc in range(DC - 1)]
        plan += [
            ((DC - 1), (DC - 1) * DC_W, 256),
            ((DC - 1), (DC - 1) * DC_W + 256, 128),
            ((DC - 1), (DC - 1) * DC_W + 384, 128),
        ]
        tile_idx = 0
        for dc, doff, dw in plan:
            if doff == dc * DC_W and dc + 2 < DC:
                load_w(dc + 2)
            uw = dw // DPN
            uoff = doff // DPN
            dsl = slice(doff, doff + dw)
            usl = slice(uoff, uoff + uw)
            woff = doff - dc * DC_W
            wsl = slice(woff, woff + dw)
            wh_sb, wl_sb = w_tiles[dc]
            for b in range(NB):
                eng = nc.vector
                tile_idx += 1
                bsl = slice(b * 128, (b + 1) * 128)
                ps = pspool.tile([128, dw], DT, name="ps")
                for k in range(KT):
                    first = k == 0
                    last = k == KT - 1
                    nc.tensor.matmul(
                        ps[:],
                        xh_sb[:, k, bsl],
                        wh_sb[:, k, wsl],
                        start=first,
                        stop=False,
                    )
                    nc.tensor.matmul(
                        ps[:],
                        xh_sb[:, k, bsl],
                        wl_sb[:, k, wsl],
                        start=False,
                        stop=False,
                    )
                    nc.tensor.matmul(
                        ps[:],
                        xl_sb[:, k, bsl],
                        wh_sb[:, k, wsl],
                        start=False,
                        stop=last,
                    )
                hc = hpool.tile([128, dw], DT, name="hc")
                # PSUM is only readable from DVE/Act: the bias-add that
                # drains PSUM stays on DVE even for gpsimd epilogues
                nc.vector.tensor_add(hc[:], ps[:], bin_bc[:, dsl])

                hc3 = hc.rearrange("p (u e) -> p u e", e=DPN)
                eng.reduce_max(m_t[b][:, usl], hc3, axis=AX)
                mb3 = (
                    m_t[b][:, usl]
                    .unsqueeze(2)
                    .broadcast_to([128, uw, DPN])
                )
                eqc = epool.tile([128, dw], DT, name="eqc")
                eng.tensor_tensor(
                    eqc.rearrange("p (u e) -> p u e", e=DPN),
                    hc3,
                    mb3,
                    op=mybir.AluOpType.is_equal,
                )
                tcw = epool.tile([128, dw], DT, name="tcw")
                eng.tensor_mul(tcw[:], eqc[:], wout_bc[:, dsl])
                sc = epool.tile([128, uw], DT, name="sc")
                tcw3 = tcw.rearrange("p (u e) -> p u e", e=DPN)
                eng.reduce_sum(sc[:], tcw3, axis=AX)
                o1 = opool.tile([128, uw], DT, name="o1")
                eng.tensor_mul(o1[:], m_t[b][:, usl], sc[:])
                o2 = opool.tile([128, uw], DT, name="o2")
                eng.tensor_add(o2[:], o1[:], bout_bc[:, usl])
                nc.scalar.dma_start(
                    out[b * 128 : (b + 1) * 128, usl], o2[:]
                )

    if split_waits:
        _split_excess_waits(nc)
    return nc


def _to_bf16(a):
    import ml_dtypes

    return a.astype(ml_dtypes.bfloat16)


def make_in_maps(x, w_in, b_in, w_out, b_out):
    x = x.astype(np.float32, copy=False)
    w_in = w_in.astype(np.float32, copy=False)
    xh = x.astype(np.float16)
    xl = _to_bf16(x - xh.astype(np.float32))
    wh = w_in.astype(np.float16)
    wl = _to_bf16(w_in - wh.astype(np.float32))

    xhT = np.ascontiguousarray(xh.T)
    xlT = np.ascontiguousarray(xl.T)
    whT = np.ascontiguousarray(wh.T)
    wlT = np.ascontiguousarray(wl.T)

    in_maps = []
    for c in range(N_CORES):
        dsl = slice(c * D_SH, (c + 1) * D_SH)
        usl = slice(c * U_SH, (c + 1) * U_SH)
        in_maps.append(
            {
                "xhT": xhT,
                "xlT": xlT,
                "whT": np.ascontiguousarray(whT[:, dsl]),
                "wlT": np.ascontiguousarray(wlT[:, dsl]),
                "bin": np.ascontiguousarray(
                    b_in[dsl].reshape(1, D_SH).astype(np.float32, copy=False)
                ),
                "wout": np.ascontiguousarray(
                    w_out[usl].reshape(1, D_SH).astype(np.float32, copy=False)
                ),
                "bout": np.ascontiguousarray(
                    b_out[usl].reshape(1, U_SH).astype(np.float32, copy=False)
                ),
            }
        )
    return in_maps


def run(in_maps, trace=False, **kw):
    nc = build_nc()
    return run_bass_kernel_spmd(
        nc, in_maps, list(range(N_CORES)), trace=trace, **kw
    )


def kernel(x, w_in, b_in, w_out, b_out):
    in_maps = make_in_maps(x, w_in, b_in, w_out, b_out)
    res = run(in_maps, trace=False)
    return np.concatenate(
        [res.results[c]["out"] for c in range(N_CORES)], axis=1
    )
